# revision 7
# baseline (speedup 1.0000x reference)
"""GraphSAGE layer (mean-aggr SAGEConv + BatchNorm1d) on 8 Trainium2 NeuronCores.

v2 strategy (dst-sharded, replicated x, batched SWDGE gathers):
  - Nodes split into 8 ranges (12500/core); each core owns all edges whose dst
    falls in its range, so aggregation completes locally. The full fp16 x is
    staged in every core's DRAM (replication = the halo exchange done at input
    staging time), so gathers never wait on a collective.
  - Edges are grouped by (dst-block of 128, src-chunk of 32768) and padded to
    multiples of 128. One dma_gather per (superblock, chunk) pulls all those
    edges' source rows in a single Pool instruction (994ns SWDGE overhead is
    amortized over ~7k rows instead of 128), bounded by the 1024-descriptor
    ring (num_idxs <= 16128, single_packet=False, int16 indices relative to
    the 32768-row chunk).
  - Per 128-edge tile, a one-hot matrix S[e,d] = (iota[e,d]==dl[e]) * w[e] is
    built on DVE and PE accumulates aggT[f,d] += G[e,f]^T @ S[e,d] in PSUM.
  - x_rawT = W_l^T@aggT + W_r^T@xT + b_l via PSUM accumulation; BN statistics
    (sum, sum-of-squares per feature) come free via scalar-engine accum_out
    during the PSUM->SBUF copies. Stats are AllReduced (tiny), and pass 2
    applies y = x_raw*scale + shift from the SBUF-resident x_rawT.
  - Outputs are written feature-major ([128, nodes]) and transposed on host.
"""

import os
from dataclasses import dataclass

import numpy as np

import concourse.bacc as bacc
import concourse.bass as bass
import concourse.mybir as mybir
import concourse.tile as tile
from concourse import library_config
from concourse.bass_utils import run_bass_kernel_spmd

F16 = mybir.dt.float16
F8 = mybir.dt.float8e4
F32 = mybir.dt.float32
I16 = mybir.dt.int16
ALU = mybir.AluOpType
ACT = mybir.ActivationFunctionType


def dma_gather_raw(nc, out_ap, in_ap, idxs_ap, num_idxs, elem_size, elem_step):
    """dma_gather with a 128-byte fp8 payload on a 256-byte row stride.
    Mirrors BassGpSimd.dma_gather but skips its elem_size_bytes%256 assert
    (verified byte-exact on hardware for this shape)."""
    gp = nc.gpsimd
    stride_bytes = elem_step * mybir.dt.size(in_ap.dtype)
    assert stride_bytes % 256 == 0
    _in_ap = gp.lower_ap_dma(in_ap, for_custom_bir_dma=True)
    _idxs_ap = gp.lower_ap(idxs_ap)
    _out_ap = gp.lower_ap(out_ap)
    return gp.add_instruction(mybir.InstDMAGatherAnt(
        name=gp.bass.get_next_instruction_name(),
        ins=[*_in_ap, _idxs_ap, gp.lower_val_access(gp.to_reg(num_idxs))],
        outs=[_out_ap],
        transpose=False,
        num_idxs=num_idxs,
        elem_size=elem_size,
        stride_bytes_256=stride_bytes // 256,
        gen_mode=0,
        single_packet=False,
        queue_num=0,
        sbuf_tokens_per_rank=0,
        sbuf_free_dim_per_rank=0,
        sbuf_free_dim_pad_per_rank=0,
        sbuf_byte_offset=0,
    ))

D = 128
P = 128
CH = 32768  # src chunk rows (int16 index range)
NQ = 4      # ceil(100000 / 32768)

LAST_EXEC_NS = None


@dataclass
class Cfg:
    N: int
    ncores: int = 8
    sb: int = 8  # dst blocks per superblock

    @property
    def npc(self):
        assert self.N % self.ncores == 0
        return self.N // self.ncores

    @property
    def nblk(self):
        return (self.npc + P - 1) // P

    @property
    def last_valid(self):
        return self.npc - (self.nblk - 1) * P

    @property
    def sblocks(self):
        return [list(range(i, min(i + self.sb, self.nblk)))
                for i in range(0, self.nblk, self.sb)]


def geometry(cfg, NT):
    """Column layout. NT[b, q] = 128-edge tiles for (dst-block b, src-chunk q).
    Within a superblock, columns are ordered chunk-major (q, then b, then
    tile), so each (superblock, chunk) gather covers one contiguous range."""
    layout = []
    col = 0
    for blocks in cfg.sblocks:
        si_c0 = col
        cw = 0
        qranges = []  # (q_start, q_end) within si
        boffs = []    # per q: {b: within-si col of (b,q) run}
        for q in range(NQ):
            qs = cw
            bo = {}
            for b in blocks:
                bo[b] = cw
                cw += int(NT[b, q])
            qranges.append((qs, cw))
            boffs.append(bo)
        layout.append(dict(si_c0=si_c0, cols=cw, qranges=qranges, boffs=boffs,
                           blocks=blocks))
        col += cw
    return layout, col


def preprocess(cfg, x, edge_index, W_l, b_l, W_r, gamma, beta):
    N, npc, nblk = cfg.N, cfg.npc, cfg.nblk
    src = np.asarray(edge_index[0], dtype=np.int64)
    dst = np.asarray(edge_index[1], dtype=np.int64)

    deg = np.bincount(dst, minlength=N)
    w_node = (1.0 / np.maximum(deg, 1.0)).astype(np.float32)

    core = dst // npc
    rel = dst - core * npc
    blk = rel >> 7
    dloc = rel & 127
    q = src >> 15
    idx_rel = (src & (CH - 1)).astype(np.int16)

    key = (core * nblk + blk) * NQ + q
    cnt = np.bincount(key, minlength=cfg.ncores * nblk * NQ)
    cnt = cnt.reshape(cfg.ncores, nblk, NQ)
    NT = (cnt.max(axis=0) + 127) // 128  # [nblk, NQ]

    layout, total_cols = geometry(cfg, NT)
    # global slot base per (b, q)
    slot_base = np.zeros((nblk, NQ), dtype=np.int64)
    for lay in layout:
        for qq in range(NQ):
            for b, off in lay["boffs"][qq].items():
                slot_base[b, qq] = (lay["si_c0"] + off) * P
    slots = total_cols * P

    order = np.argsort(key, kind="stable")
    ks = key[order]
    grp_first = np.r_[0, np.flatnonzero(np.diff(ks)) + 1]
    starts = np.zeros(ks.shape[0], dtype=np.int64)
    starts[grp_first] = grp_first
    starts = np.maximum.accumulate(starts)
    rank = np.arange(ks.shape[0], dtype=np.int64) - starts

    bounds = np.searchsorted(ks, np.arange(cfg.ncores + 1) * (nblk * NQ))
    per_core = []
    for c in range(cfg.ncores):
        a, bnd = bounds[c], bounds[c + 1]
        ecs = order[a:bnd]
        kk = ks[a:bnd] - c * (nblk * NQ)
        slot = slot_base[kk // NQ, kk % NQ] + rank[a:bnd]

        a_idx = np.zeros(slots, dtype=np.int16)
        a_dl = np.full(slots, -1.0, dtype=np.float16)
        a_wv = np.zeros(slots, dtype=np.float16)
        a_idx[slot] = idx_rel[ecs]
        a_dl[slot] = dloc[ecs].astype(np.float16)
        a_wv[slot] = w_node[dst[ecs]].astype(np.float16)

        # idx16 [128, total_cols*8]: per gather g (si,q) with global column
        # range [g0, g1), element i at [i%16 (+16r), g0*8 + i//16].
        idx16 = np.zeros((P, total_cols * 8), dtype=np.int16)
        for lay in layout:
            for qq in range(NQ):
                qs, qe = lay["qranges"][qq]
                if qe == qs:
                    continue
                g0, g1 = lay["si_c0"] + qs, lay["si_c0"] + qe
                flat = a_idx[g0 * P:g1 * P]
                wrapped = flat.reshape(-1, 16).T  # [16, (g1-g0)*8]
                idx16[:, g0 * 8:g1 * 8] = np.tile(wrapped, (8, 1))

        # dl/wv: slot s at [s%128, s//128]
        dl_t = np.ascontiguousarray(a_dl.reshape(-1, P).T)
        wv_t = np.ascontiguousarray(a_wv.reshape(-1, P).T)

        xT = np.zeros((P, nblk * P), dtype=np.float16)
        xT[:, :npc] = np.asarray(x[c * npc:(c + 1) * npc], dtype=np.float16).T

        per_core.append(dict(idx16=idx16, dl=dl_t, wv=wv_t, xT=xT))

    import ml_dtypes
    x8 = np.zeros((N, 256), dtype=ml_dtypes.float8_e4m3)
    x8[:, :128] = np.asarray(x, dtype=ml_dtypes.float8_e4m3)
    shared = dict(
        x8=x8,
        wl=np.asarray(W_l, dtype=np.float16),
        wr=np.asarray(W_r, dtype=np.float16),
        blr=np.asarray(b_l, dtype=np.float16).reshape(1, D),
        gamma=np.asarray(gamma, dtype=np.float32).reshape(P, 1),
        beta=np.asarray(beta, dtype=np.float32).reshape(P, 1),
        iota=np.tile(np.arange(P, dtype=np.float16), (P, 1)),
    )
    return NT, per_core, shared


def build_program(cfg, NT):
    layout, total_cols = geometry(cfg, NT)
    N, nblk, npc = cfg.N, cfg.nblk, cfg.npc

    nc = bacc.Bacc("TRN2", target_bir_lowering=False, debug=False,
                   num_devices=cfg.ncores)
    x8_d = nc.dram_tensor("x8", [N, 256], F8, kind="ExternalInput").ap()
    idx_d = nc.dram_tensor("idx16", [P, total_cols * 8], I16,
                           kind="ExternalInput").ap()
    dl_d = nc.dram_tensor("dl", [P, total_cols], F16, kind="ExternalInput").ap()
    wv_d = nc.dram_tensor("wv", [P, total_cols], F16, kind="ExternalInput").ap()
    xT_d = nc.dram_tensor("xT", [P, nblk * P], F16, kind="ExternalInput").ap()
    wl_d = nc.dram_tensor("wl", [D, D], F16, kind="ExternalInput").ap()
    wr_d = nc.dram_tensor("wr", [D, D], F16, kind="ExternalInput").ap()
    blr_d = nc.dram_tensor("blr", [1, D], F16, kind="ExternalInput").ap()
    gamma_d = nc.dram_tensor("gamma", [P, 1], F32, kind="ExternalInput").ap()
    beta_d = nc.dram_tensor("beta", [P, 1], F32, kind="ExternalInput").ap()
    iota_d = nc.dram_tensor("iota", [P, P], F16, kind="ExternalInput").ap()
    xraw_d = nc.dram_tensor("xrawT", [P, nblk * P], F16, kind="ExternalOutput").ap()
    xdesk_d = nc.dram_tensor("xdeskT", [P, nblk * P], F16, kind="ExternalOutput").ap()

    with tile.TileContext(nc) as tc:
        from contextlib import ExitStack
        with ExitStack() as ctx:
            cpool = ctx.enter_context(tc.tile_pool(name="const", bufs=1))
            gpool = ctx.enter_context(tc.tile_pool(name="gbuf", bufs=2))
            ipool = ctx.enter_context(tc.tile_pool(name="ibuf", bufs=3))
            mpool = ctx.enter_context(tc.tile_pool(name="meta", bufs=3))
            spool = ctx.enter_context(tc.tile_pool(name="stile", bufs=3))
            apool = ctx.enter_context(tc.tile_pool(name="aggT", bufs=4))
            sqp = ctx.enter_context(tc.tile_pool(name="sq", bufs=2))
            ppool = ctx.enter_context(tc.tile_pool(name="parts", bufs=6))
            psA = ctx.enter_context(tc.tile_pool(name="psA", bufs=3, space="PSUM"))
            psB = ctx.enter_context(tc.tile_pool(name="psB", bufs=3, space="PSUM"))
            dpool = ctx.enter_context(tc.tile_pool(name="dram", bufs=1, space="DRAM"))

            nc.gpsimd.load_library(library_config.mlp)

            iota_sb = cpool.tile([P, P], F16)
            wl_sb = cpool.tile([D, D], F16)
            wr_sb = cpool.tile([D, D], F16)
            blr_sb = cpool.tile([1, D], F16)
            gamma_sb = cpool.tile([P, 1], F32)
            beta_sb = cpool.tile([P, 1], F32)
            ones_sb = cpool.tile([1, P], F16)
            sum_acc = cpool.tile([P, 1], F32)
            ssq_acc = cpool.tile([P, 1], F32)
            sstrip = cpool.tile([P, nblk], F32)
            qstrip = cpool.tile([P, nblk], F32)
            xt_sb = cpool.tile([P, nblk * P], F16)
            stg = cpool.tile([P, nblk * P], F16)
            nc.sync.dma_start(iota_sb[:], iota_d[:])
            nc.sync.dma_start(wl_sb[:], wl_d[:])
            nc.sync.dma_start(wr_sb[:], wr_d[:])
            nc.sync.dma_start(blr_sb[:], blr_d[:])
            nc.sync.dma_start(gamma_sb[:], gamma_d[:])
            nc.sync.dma_start(beta_sb[:], beta_d[:])
            nc.sync.dma_start(xt_sb[:], xT_d[:])
            nc.vector.memset(ones_sb[:], 1.0)
            nc.vector.memset(sum_acc[:], 0.0)
            nc.vector.memset(ssq_acc[:], 0.0)

            for si, lay in enumerate(layout):
                blocks, si_c0, si_cols = lay["blocks"], lay["si_c0"], lay["cols"]
                if si_cols == 0:
                    continue
                idx_sb = ipool.tile([P, si_cols * 8], I16, tag="i")
                nc.sync.dma_start(idx_sb[:], idx_d[:, si_c0 * 8:(si_c0 + si_cols) * 8])
                dl16 = mpool.tile([P, si_cols], F16, tag="dl16")
                wv16 = mpool.tile([P, si_cols], F16, tag="wv16")
                dl_sb = mpool.tile([P, si_cols], F32, tag="dl")
                wv_sb = mpool.tile([P, si_cols], F32, tag="wv")
                nc.sync.dma_start(dl16[:], dl_d[:, si_c0:si_c0 + si_cols])
                nc.sync.dma_start(wv16[:], wv_d[:, si_c0:si_c0 + si_cols])
                nc.vector.tensor_copy(dl_sb[:], dl16[:])
                nc.vector.tensor_copy(wv_sb[:], wv16[:])

                gbuf = gpool.tile([P, si_cols, P], F8, tag="g")
                for qq in range(NQ):
                    qs, qe = lay["qranges"][qq]
                    if qe == qs:
                        continue
                    qlo = qq * CH
                    qhi = min((qq + 1) * CH, N)
                    dma_gather_raw(
                        nc, gbuf[:, qs:qe, :], x8_d[qlo:qhi, 0:D],
                        idx_sb[:, qs * 8:qe * 8],
                        num_idxs=(qe - qs) * P, elem_size=D, elem_step=256,
                    )

                for b in blocks:
                    valid = cfg.last_valid if b == nblk - 1 else P
                    tiles = []
                    for qq in range(NQ):
                        bo = lay["boffs"][qq][b]
                        tiles += [bo + t for t in range(int(NT[b, qq]))]
                    pa = psA.tile([P, P], F32, tag="pa", space="PSUM")
                    if not tiles:
                        aggT = apool.tile([P, P], F16, tag="a")
                        nc.vector.memset(aggT[:], 0.0)
                    else:
                        stw = spool.tile([P, len(tiles), P], F16, tag="s")
                        for ti, col in enumerate(tiles):
                            nc.vector.tensor_scalar(
                                stw[:, ti, :], iota_sb[:],
                                dl_sb[:, col:col + 1], wv_sb[:, col:col + 1],
                                ALU.is_equal, ALU.mult,
                            )
                        for ti, col in enumerate(tiles):
                            nc.tensor.matmul(
                                out=pa[:], lhsT=gbuf[:, col:col + 1, :],
                                rhs=stw[:, ti, :],
                                start=(ti == 0), stop=(ti == len(tiles) - 1),
                            )
                        aggT = apool.tile([P, P], F16, tag="a")
                        nc.scalar.activation(aggT[:], pa[:], ACT.Copy)

                    pb = psB.tile([P, P], F32, tag="pb", space="PSUM")
                    nc.tensor.matmul(out=pb[:], lhsT=wl_sb[:], rhs=aggT[:],
                                     start=True, stop=False)
                    nc.tensor.matmul(out=pb[:], lhsT=wr_sb[:],
                                     rhs=xt_sb[:, b * P:(b + 1) * P],
                                     start=False, stop=False)
                    nc.tensor.matmul(out=pb[:], lhsT=blr_sb[:], rhs=ones_sb[:],
                                     start=False, stop=True)

                    sq = sqp.tile([P, P], F32, tag="sq")
                    nc.scalar.activation(stg[:, b * P:b * P + valid],
                                         pb[:, :valid], ACT.Copy,
                                         accum_out=sstrip[:, b:b + 1])
                    nc.scalar.activation(sq[:, :valid], pb[:, :valid], ACT.Square,
                                         accum_out=qstrip[:, b:b + 1])

                c0 = blocks[0] * P
                sbvalid = (len(blocks) - 1) * P + (cfg.last_valid
                          if blocks[-1] == nblk - 1 else P)
                nc.sync.dma_start(xraw_d[:, c0:c0 + sbvalid], stg[:, c0:c0 + sbvalid])

            # ---- BN stats all-reduce + scale/shift ----
            nc.vector.tensor_reduce(sum_acc[:], sstrip[:], mybir.AxisListType.X,
                                    ALU.add)
            nc.vector.tensor_reduce(ssq_acc[:], qstrip[:], mybir.AxisListType.X,
                                    ALU.add)
            stats = cpool.tile([P, 2], F32)
            nc.vector.tensor_copy(stats[:, 0:1], sum_acc[:])
            nc.vector.tensor_copy(stats[:, 1:2], ssq_acc[:])
            cc_in = dpool.tile([P, 2], F32)
            cc_out = dpool.tile([cfg.ncores * P, 2], F32)
            nc.sync.dma_start(cc_in[:], stats[:])
            # AllGather + local sum is ~2x cheaper than AllReduce in the
            # collective cost model (no 1.875x reduce multiplier). The
            # gather concatenates along partitions: core c's stats land at
            # rows [c*128, (c+1)*128).
            nc.gpsimd.collective_compute(
                "AllGather", ALU.bypass,
                replica_groups=[list(range(cfg.ncores))],
                ins=[cc_in.opt()], outs=[cc_out.opt()],
            )
            astats = cpool.tile([P, cfg.ncores, 2], F32)
            for c in range(cfg.ncores):
                nc.sync.dma_start(astats[:, c, :], cc_out[c * P:(c + 1) * P, :])
            gstats = cpool.tile([P, 2], F32)
            nc.vector.tensor_reduce(gstats[:, 0:1], astats[:, :, 0],
                                    mybir.AxisListType.X, ALU.add)
            nc.vector.tensor_reduce(gstats[:, 1:2], astats[:, :, 1],
                                    mybir.AxisListType.X, ALU.add)

            mean = cpool.tile([P, 1], F32)
            ex2 = cpool.tile([P, 1], F32)
            var = cpool.tile([P, 1], F32)
            std = cpool.tile([P, 1], F32)
            rstd = cpool.tile([P, 1], F32)
            scl = cpool.tile([P, 1], F32)
            sft = cpool.tile([P, 1], F32)
            tmp = cpool.tile([P, 1], F32)
            inv_n = 1.0 / float(N)
            nc.vector.tensor_scalar(mean[:], gstats[:, 0:1], inv_n, None, ALU.mult)
            nc.vector.tensor_scalar(ex2[:], gstats[:, 1:2], inv_n, None, ALU.mult)
            nc.vector.tensor_tensor(tmp[:], mean[:], mean[:], ALU.mult)
            nc.vector.tensor_tensor(var[:], ex2[:], tmp[:], ALU.subtract)
            nc.vector.tensor_scalar(var[:], var[:], 1e-5, None, ALU.add)
            nc.scalar.activation(std[:], var[:], ACT.Sqrt)
            nc.vector.reciprocal(rstd[:], std[:])
            nc.vector.tensor_tensor(scl[:], rstd[:], gamma_sb[:], ALU.mult)
            nc.vector.tensor_tensor(tmp[:], mean[:], scl[:], ALU.mult)
            nc.vector.tensor_tensor(sft[:], beta_sb[:], tmp[:], ALU.subtract)

            # ---- pass 2: normalize from SBUF-resident stg ----
            p2 = ctx.enter_context(tc.tile_pool(name="p2", bufs=2))
            for lay in layout:
                blocks = lay["blocks"]
                c0 = blocks[0] * P
                sbvalid = (len(blocks) - 1) * P + (cfg.last_valid
                          if blocks[-1] == nblk - 1 else P)
                xd = p2.tile([P, sbvalid], F16, tag="xd")
                nc.vector.tensor_scalar(xd[:], stg[:, c0:c0 + sbvalid],
                                        scl[:], sft[:], ALU.mult, ALU.add)
                nc.sync.dma_start(xdesk_d[:, c0:c0 + sbvalid], xd[:])

    nc.compile()
    return nc


_CACHE = {}


def _child_worker(conn, args):
    try:
        out = run_graph(*args, _allow_subprocess=False)
        conn.send(("ok", out))
    except BaseException as e:  # noqa: BLE001
        conn.send(("err", repr(e)))
    finally:
        conn.close()


def _run_in_subprocess(args):
    import multiprocessing as mp
    ctx = mp.get_context("spawn")
    parent, child = ctx.Pipe()
    p = ctx.Process(target=_child_worker, args=(child, args))
    p.start()
    status, payload = parent.recv()
    p.join()
    if status != "ok":
        raise RuntimeError(f"subprocess kernel run failed: {payload}")
    return payload


def run_graph(x, edge_index, W_l, b_l, W_r, gamma, beta, ncores=8, trace=False,
              _allow_subprocess=True):
    global LAST_EXEC_NS
    x = np.asarray(x, dtype=np.float32)
    N = x.shape[0]
    cfg = Cfg(N=N, ncores=ncores)
    NT, per_core, shared = preprocess(cfg, x, edge_index, W_l, b_l, W_r,
                                      gamma, beta)

    key = (N, ncores, NT.tobytes())
    if key not in _CACHE:
        _CACHE[key] = build_program(cfg, NT)
    nc = _CACHE[key]

    in_maps = []
    for c in range(ncores):
        m = dict(shared)
        m.update(per_core[c])
        in_maps.append(m)

    try:
        res = run_bass_kernel_spmd(nc, in_maps, core_ids=list(range(ncores)),
                                   trace=trace)
    except Exception:
        if not _allow_subprocess:
            raise
        args = (x, edge_index, W_l, b_l, W_r, gamma, beta, ncores, trace)
        for attempt in range(4):
            try:
                return _run_in_subprocess(args)
            except Exception:
                if attempt == 3:
                    raise
                import time as _t
                _t.sleep(45)
    LAST_EXEC_NS = res.exec_time_ns

    npc = cfg.npc
    xraw = np.empty((N, D), dtype=np.float32)
    xdesk = np.empty((N, D), dtype=np.float32)
    for c in range(ncores):
        xraw[c * npc:(c + 1) * npc] = res.results[c]["xrawT"][:, :npc].T.astype(np.float32)
        xdesk[c * npc:(c + 1) * npc] = res.results[c]["xdeskT"][:, :npc].T.astype(np.float32)
    return xraw, xdesk


def kernel(x, edge_index, W_l, b_l, W_r, gamma, beta):
    return run_graph(np.asarray(x), np.asarray(edge_index), np.asarray(W_l),
                     np.asarray(b_l), np.asarray(W_r), np.asarray(gamma),
                     np.asarray(beta), ncores=8,
                     trace=bool(int(os.environ.get("KERNEL_TRACE", "0"))))
